# revision 13
# baseline (speedup 1.0000x reference)
"""GNN message-passing (mean aggregation + dual linear + relu + L2 norm)
on 8 Trainium2 NeuronCores.

Strategy (dst-sharded, single-pass, pair-packed gather, 2 overlapping
int16 windows):
  - Nodes are globally sorted by in-degree and dealt round-robin to the 8
    cores (SPMD: identical compiled schedule per core).
  - h_neigh is presented to the device as fp16 PAIR rows: row i =
    [h[2i], h[2i+1]] -> [50000, 128] fp16.  50000 rows exceed the int16
    gather-index range, so two OVERLAPPING row windows are used:
    A = rows [0, 32767), B = rows [17233, 50000).  Pairs in the overlap
    [17233, 32767) are routed per node to balance its A/B slot counts, so
    one shared node order (sorted by the (a, b) slot-count tuple) keeps
    per-tile padding low (~6%) for BOTH windows -> no combine stage, no
    HBM partial bounce.
  - Per 128-node tile: one gather per window fetches [128, K, 128] fp16
    (256-byte descriptors, one per edge-pair-slot).  A per-half fp16 count
    mask zeroes the unwanted half of each pair (and padding); an in-place
    contiguous binary-tree of DVE adds produces the neighbor sums (fp32).
  - Epilogue per tile: scale by 1/deg, PE transpose, dual matmul with
    W_neigh/W_self (PSUM-accumulated), relu, transpose back, fused
    square+row-sum, sqrt, reciprocal, scale, DMA out.
  - CPU does only integer index prep (sorting, routing, dedup) plus input
    layout (fp16 cast/reshape of h_neigh, transpose of h_self, 1/deg) and
    the final row unpermute of the outputs.
"""
import numpy as np
from contextlib import ExitStack

N_NODES = 100000
N_EDGES = 1600000
D = 64
N_CORES = 8
NPC = 12544                 # nodes per core (98 tiles of 128)
NT = NPC // 128             # 98 output tiles per core
PAIRS = 50000               # pair rows in the fp16 table
W_LO = 17233                # window B start (A = [0, 32767), B = [W_LO, 50000))
W_HI = 32767
CALL_COLS = 56              # target gather-call width per window

_cache = {}


def _prep(h_neigh, h_self, src, dst, W_neigh, W_self):
    """CPU-side integer/index preprocessing. Returns (in_maps, sched, meta)."""
    src = np.asarray(src, dtype=np.int64)
    dst = np.asarray(dst, dtype=np.int64)
    h_neigh = np.asarray(h_neigh, dtype=np.float32)
    h_self = np.asarray(h_self, dtype=np.float32)

    deg = np.bincount(dst, minlength=N_NODES)                    # [N]
    order = np.argsort(-deg, kind="stable")                      # degree sort
    n_ext = NPC * N_CORES                                        # 100352
    order_ext = np.concatenate(
        [order, np.full(n_ext - N_NODES, N_NODES, dtype=np.int64)])
    deg_ext = np.concatenate([deg, [0]])
    inv_deg = 1.0 / np.maximum(deg_ext, 1).astype(np.float32)

    rank_of_node = np.empty(N_NODES + 1, dtype=np.int64)
    rank_of_node[order_ext[:n_ext]] = np.arange(n_ext)

    e_rank = rank_of_node[dst]
    e_core = e_rank % N_CORES
    e_pos = e_rank // N_CORES                  # 0..NPC-1
    e_pair = src >> 1
    e_par = src & 1

    tbl16 = np.ascontiguousarray(
        h_neigh.astype(np.float16).reshape(PAIRS, 2 * D))

    # per core: route each edge to window A or B (balance per node using
    # the flexible overlap), dedup (pos, window, relpair) into slots
    per_core = []
    KA = np.zeros((N_CORES, NT), dtype=np.int64)
    KB = np.zeros((N_CORES, NT), dtype=np.int64)
    vorders = []
    for c in range(N_CORES):
        m = e_core == c
        pos = e_pos[m]
        pair = e_pair[m]
        par = e_par[m]
        fa = np.bincount(pos[pair < W_LO], minlength=NPC)
        fl = np.bincount(pos[(pair >= W_LO) & (pair < W_HI)], minlength=NPC)
        d = np.bincount(pos, minlength=NPC)
        na = np.clip((d + 1) // 2, fa, fa + fl)     # A-routed edge count
        # route: forced-A -> A; forced-B -> B; flexible: first (na-fa)
        # per node (in sorted order) -> A
        so = np.lexsort((pair, pos))
        pos_s = pos[so]; pair_s = pair[so]; par_s = par[so]
        flex = (pair_s >= W_LO) & (pair_s < W_HI)
        # rank of each flexible edge within its node's flexible list
        idxf = np.flatnonzero(flex)
        posf = pos_s[idxf]
        first = np.searchsorted(posf, np.arange(NPC), side="left")
        frank = np.arange(idxf.size) - first[posf]
        toA = np.zeros(pos_s.size, dtype=bool)
        toA[pair_s < W_LO] = True
        toA[idxf[frank < (na - fa)[posf]]] = True
        # dedup (pos, window, relpair)
        win = (~toA).astype(np.int64)              # 0 = A, 1 = B
        relp = pair_s - win * W_LO
        key = (pos_s * 2 + win) * PAIRS + relp
        uk, inv = np.unique(key, return_inverse=True)
        cnt = np.zeros((uk.size, 2), dtype=np.int64)
        np.add.at(cnt, (inv, par_s), 1)
        upos = uk // (2 * PAIRS)
        uwin = (uk // PAIRS) % 2
        urel = uk % PAIRS
        a_slots = np.bincount(upos[uwin == 0], minlength=NPC)
        b_slots = np.bincount(upos[uwin == 1], minlength=NPC)
        vo = np.lexsort((-a_slots, -b_slots))
        vorders.append(vo)
        vrank = np.empty(NPC, dtype=np.int64)
        vrank[vo] = np.arange(NPC)
        KA[c] = a_slots[vo].reshape(NT, 128).max(axis=1)
        KB[c] = b_slots[vo].reshape(NT, 128).max(axis=1)
        per_core.append(dict(upos=upos, uwin=uwin, urel=urel, cnt=cnt,
                             vrank=vrank))

    KAm = KA.max(axis=0)
    KBm = KB.max(axis=0)
    SKA = int(KAm.sum())
    SKB = int(KBm.sum())
    colA = np.zeros(NT + 1, dtype=np.int64); colA[1:] = np.cumsum(KAm)
    colB = np.zeros(NT + 1, dtype=np.int64); colB[1:] = np.cumsum(KBm)

    # call grouping: consecutive tiles while BOTH windows fit the cap;
    # taper the final tiles so the tail drain + compute chain is short
    calls = []                      # (t0, t1, colA0, nA, colB0, nB)
    t = 0
    while t < NT:
        cap = CALL_COLS if t < NT - 12 else 20
        t1 = t
        ca = cb = 0
        while t1 < NT and (ca == 0 or
                           (ca + KAm[t1] <= cap and
                            cb + KBm[t1] <= cap)):
            ca += KAm[t1]; cb += KBm[t1]
            t1 += 1
        calls.append((t, t1, int(colA[t]), int(ca), int(colB[t]), int(cb)))
        t = t1
    sched = dict(KAm=KAm.tolist(), KBm=KBm.tolist(), SKA=SKA, SKB=SKB,
                 calls=calls)

    Wn_T = np.ascontiguousarray(W_neigh.astype(np.float32).T)
    Ws_T = np.ascontiguousarray(W_self.astype(np.float32).T)
    wT = np.concatenate([Wn_T, Ws_T], axis=1)                   # [64, 128]
    h_self_ext = np.vstack([h_self, np.zeros((1, D), np.float32)])

    def wrap(M):
        flat = M.T.reshape(-1)
        w = flat.reshape(-1, 16).T
        return np.tile(w, (8, 1)).copy()

    in_maps = []
    core_nodes = []
    for c in range(N_CORES):
        pc = per_core[c]
        upos, uwin, urel, cnt, vrank = (pc["upos"], pc["uwin"], pc["urel"],
                                        pc["cnt"], pc["vrank"])
        vp = vrank[upos]
        part = vp % 128
        tile = vp // 128
        # slot index within (node, window): uk sorted by (pos, win, relp)
        # -> recompute per window
        idxA = np.zeros((128, SKA), dtype=np.int16)
        mskA = np.zeros((128, 2 * SKA), dtype=np.float16)
        idxB = np.zeros((128, SKB), dtype=np.int16)
        mskB = np.zeros((128, 2 * SKB), dtype=np.float16)
        for w, (idxW, mskW, colW) in enumerate(
                ((idxA, mskA, colA), (idxB, mskB, colB))):
            sel = uwin == w
            vpw = vp[sel]; partw = part[sel]; tilew = tile[sel]
            relw = urel[sel]; cntw = cnt[sel]
            # slot rank within node for this window (uk order is sorted by
            # key = (pos*2+win)*PAIRS+relp, so within (pos, win) ascending)
            so = np.lexsort((relw, vpw))
            vpw = vpw[so]; partw = partw[so]; tilew = tilew[so]
            relw = relw[so]; cntw = cntw[so]
            first = np.searchsorted(vpw, np.arange(NPC), side="left")
            srank = np.arange(vpw.size) - first[vpw]
            col = colW[tilew] + srank
            idxW[partw, col] = relw.astype(np.int16)
            mskW[partw, 2 * col] = cntw[:, 0].astype(np.float16)
            mskW[partw, 2 * col + 1] = cntw[:, 1].astype(np.float16)

        nodes = order_ext[c::N_CORES][vorders[c]]
        core_nodes.append(nodes)
        hsT = np.ascontiguousarray(h_self_ext[nodes].T)          # [64, NPC]
        ivd = inv_deg[nodes].reshape(NT, 128).T.copy()           # [128, NT]

        in_maps.append(dict(tbl=tbl16, idxA=wrap(idxA), mskA=mskA,
                            idxB=wrap(idxB), mskB=mskB, hsT=hsT,
                            ivd=ivd, wT=wT))

    meta = dict(core_nodes=core_nodes)
    return in_maps, sched, meta


def _patch_queue_aware_sems():
    """Make Tile's DMASW sem-lane assignment follow each SWDGE instruction's
    queue_num (ucode requires a sem to be updated from a single queue)."""
    from concourse import tile_sem_assignment as tsa
    from concourse import mybir
    if getattr(tsa.TileClockTick, "_qaware_patched", False):
        return
    orig = tsa.TileClockTick._assign_tick

    def _assign_tick_qaware(self, inst):
        qn = getattr(inst, "queue_num", None)
        if qn is not None and getattr(inst, "engine", None) == mybir.EngineType.Pool:
            self.next_sw_dma_idx = int(qn) % self.swdge_sem_count
        return orig(self, inst)

    tsa.TileClockTick._assign_tick = _assign_tick_qaware
    tsa.TileClockTick._qaware_patched = True


def _build(sched):
    import concourse.bacc as bacc
    import concourse.tile as tile
    from concourse import mybir
    from concourse.masks import make_identity

    _patch_queue_aware_sems()

    F32 = mybir.dt.float32
    F16 = mybir.dt.float16
    I16 = mybir.dt.int16
    AF = mybir.ActivationFunctionType
    KAm = np.array(sched["KAm"])
    KBm = np.array(sched["KBm"])
    SKA = int(sched["SKA"]); SKB = int(sched["SKB"])
    calls = sched["calls"]
    colA = np.zeros(NT + 1, dtype=np.int64); colA[1:] = np.cumsum(KAm)
    colB = np.zeros(NT + 1, dtype=np.int64); colB[1:] = np.cumsum(KBm)

    nc = bacc.Bacc("TRN2", target_bir_lowering=False, num_swdge_queues=4,
                   dynamic_dma_scratch_size=32768)
    tbl = nc.declare_dram_parameter("tbl", [PAIRS, 2 * D], F16, isOutput=False)
    idxA = nc.declare_dram_parameter("idxA", [128, SKA * 8], I16, isOutput=False)
    mskA = nc.declare_dram_parameter("mskA", [128, 2 * SKA], F16, isOutput=False)
    idxB = nc.declare_dram_parameter("idxB", [128, SKB * 8], I16, isOutput=False)
    mskB = nc.declare_dram_parameter("mskB", [128, 2 * SKB], F16, isOutput=False)
    hsT = nc.declare_dram_parameter("hsT", [D, NPC], F32, isOutput=False)
    ivd = nc.declare_dram_parameter("ivd", [128, NT], F32, isOutput=False)
    wT = nc.declare_dram_parameter("wT", [D, 2 * D], F32, isOutput=False)
    out = nc.declare_dram_parameter("out", [NPC, D], F32, isOutput=True)

    with tile.TileContext(nc) as tc, ExitStack() as ctx:
        singles = ctx.enter_context(tc.tile_pool(name="singles", bufs=1))
        ld = ctx.enter_context(tc.tile_pool(name="load", bufs=4))
        gpA = ctx.enter_context(tc.tile_pool(name="gathA", bufs=4))
        gpB = ctx.enter_context(tc.tile_pool(name="gathB", bufs=4))
        wk = ctx.enter_context(tc.tile_pool(name="work", bufs=3))
        ps = ctx.enter_context(tc.tile_pool(name="psum", bufs=2, space="PSUM"))

        ivd_sb = singles.tile([128, NT], F32)
        nc.sync.dma_start(out=ivd_sb[:], in_=ivd[:])
        wT_sb = singles.tile([D, 2 * D], F32)
        nc.sync.dma_start(out=wT_sb[:], in_=wT[:])
        # h_self^T loaded in 4 tile-aligned chunks (epilogue is tile-ordered)
        HS_CH = 4
        hs_tiles = [0, 25, 50, 74, NT]
        hs_bound = [b * 128 for b in hs_tiles]
        hsT_sb = [singles.tile([D, hs_bound[i + 1] - hs_bound[i]], F32,
                               name=f"hsT{i}")
                  for i in range(HS_CH)]
        ident = singles.tile([128, 128], F32)
        make_identity(nc, ident[:])
        eps = singles.tile([128, 1], F32)
        nc.gpsimd.memset(eps[:], 1e-30)

        def load_hs(i):
            nc.sync.dma_start(out=hsT_sb[i][:],
                              in_=hsT[:, hs_bound[i]:hs_bound[i + 1]])

        load_hs(0)

        def hs_slice(t):
            """[64, 128] h_self^T slice for tile t."""
            i = next(j for j in range(HS_CH) if t < hs_tiles[j + 1])
            lo = (t - hs_tiles[i]) * 128
            return hsT_sb[i][:, lo:lo + 128]

        def load_call(k):
            """JIT-load idx+mask slices for call k; returns sbuf tiles."""
            (t0, t1, cA0, nA, cB0, nB) = calls[k]
            r = {}
            if nA > 0:
                ia = ld.tile([128, nA * 8], I16, tag="iA")
                nc.sync.dma_start(out=ia[:],
                                  in_=idxA[:, cA0 * 8:(cA0 + nA) * 8])
                ma = ld.tile([128, 2 * nA], F16, tag="mA")
                nc.sync.dma_start(out=ma[:], in_=mskA[:, 2 * cA0:2 * (cA0 + nA)])
                r["A"] = (ia, ma)
            if nB > 0:
                ib = ld.tile([128, nB * 8], I16, tag="iB")
                nc.sync.dma_start(out=ib[:],
                                  in_=idxB[:, cB0 * 8:(cB0 + nB) * 8])
                mb = ld.tile([128, 2 * nB], F16, tag="mB")
                nc.sync.dma_start(out=mb[:], in_=mskB[:, 2 * cB0:2 * (cB0 + nB)])
                r["B"] = (ib, mb)
            return r

        qn = [0]
        def next_q():
            q = qn[0] % 4
            qn[0] += 1
            return q

        def do_tile(t, red):
            agg = wk.tile([128, D], F32, tag="agg")
            nc.scalar.mul(agg[:], red[:], ivd_sb[:, t:t + 1])
            p_aT = ps.tile([D, 128], F32, tag="aT")
            nc.tensor.transpose(out=p_aT[:], in_=agg[:], identity=ident[:])
            aT = wk.tile([D, 128], F32, tag="aTs")
            nc.vector.tensor_copy(aT[:], p_aT[:])
            p_z = ps.tile([D, 128], F32, tag="z")
            nc.tensor.matmul(out=p_z[:], lhsT=wT_sb[:, 0:D], rhs=aT[:],
                             start=True, stop=False)
            nc.tensor.matmul(out=p_z[:], lhsT=wT_sb[:, D:2 * D],
                             rhs=hs_slice(t),
                             start=False, stop=True)
            zT = wk.tile([D, 128], F32, tag="zT")
            nc.scalar.activation(zT[:], p_z[:], AF.Relu)
            p_zn = ps.tile([128, D], F32, tag="zn")
            nc.tensor.transpose(out=p_zn[:], in_=zT[:],
                                identity=ident[0:D, 0:D])
            sq = wk.tile([128, D], F32, tag="sq")
            s = wk.tile([128, 1], F32, tag="s")
            nc.scalar.activation(sq[:], p_zn[:], AF.Square, accum_out=s[:])
            nrm = wk.tile([128, 1], F32, tag="nrm")
            nc.scalar.activation(nrm[:], s[:], AF.Sqrt, bias=eps[:])
            r = wk.tile([128, 1], F32, tag="r")
            nc.vector.reciprocal(r[:], nrm[:])
            o = wk.tile([128, D], F32, tag="o")
            nc.scalar.mul(o[:], p_zn[:], r[:])
            nc.sync.dma_start(out=out[t * 128:(t + 1) * 128, :], in_=o[:])

        def tree(gt, cols2):
            """In-place contiguous binary-tree sum over [128, cols2, 64]
            view `gt`; returns after leaving the total in cols 0..1."""
            cur = cols2
            while cur > 2:
                h = cur // 2
                odd = cur - 2 * h
                nc.vector.tensor_add(
                    gt[:, odd:odd + h, :],
                    gt[:, odd:odd + h, :],
                    gt[:, odd + h:cur, :])
                cur = odd + h
            return cur       # 2 (cols2 >= 2 always) or 1

        PREFETCH = 2
        loaded = {}
        for k in range(min(PREFETCH, len(calls))):
            loaded[k] = load_call(k)

        for ci, (t0, t1, cA0, nA, cB0, nB) in enumerate(calls):
            if ci + PREFETCH < len(calls):
                loaded[ci + PREFETCH] = load_call(ci + PREFETCH)
            if ci < HS_CH - 1:
                load_hs(ci + 1)
            tiles_ld = loaded.pop(ci)
            gA = gB = None
            if nA > 0:
                ia, ma = tiles_ld["A"]
                gA = gpA.tile([128, nA, 2 * D], F16, tag="gA")
                nc.gpsimd.dma_gather(
                    out_ap=gA[:], in_ap=tbl[0:W_HI, :],
                    idxs_ap=ia[:],
                    num_idxs=nA * 128, num_idxs_reg=nA * 128,
                    elem_size=2 * D, single_packet=False,
                    queue_num=next_q())
                gvA = gA[:].rearrange("p c (s d) -> p (c s) d", s=2, d=D)
                nc.vector.tensor_mul(
                    gvA, gvA, ma[:, :, None].broadcast_to((128, 2 * nA, D)))
            if nB > 0:
                ib, mb = tiles_ld["B"]
                gB = gpB.tile([128, nB, 2 * D], F16, tag="gB")
                nc.gpsimd.dma_gather(
                    out_ap=gB[:], in_ap=tbl[W_LO:PAIRS, :],
                    idxs_ap=ib[:],
                    num_idxs=nB * 128, num_idxs_reg=nB * 128,
                    elem_size=2 * D, single_packet=False,
                    queue_num=next_q())
                gvB = gB[:].rearrange("p c (s d) -> p (c s) d", s=2, d=D)
                nc.vector.tensor_mul(
                    gvB, gvB, mb[:, :, None].broadcast_to((128, 2 * nB, D)))
            for t in range(t0, t1):
                ka = int(KAm[t]); kb = int(KBm[t])
                red = wk.tile([128, D], F32, tag="red")
                parts = []
                if ka > 0:
                    la = int(colA[t]) - cA0
                    ga_t = gA[:, la:la + ka, :].rearrange(
                        "p c (s d) -> p (c s) d", s=2, d=D)
                    tree(ga_t, 2 * ka)
                    parts.append(ga_t)
                if kb > 0:
                    lb = int(colB[t]) - cB0
                    gb_t = gB[:, lb:lb + kb, :].rearrange(
                        "p c (s d) -> p (c s) d", s=2, d=D)
                    tree(gb_t, 2 * kb)
                    parts.append(gb_t)
                if len(parts) == 2:
                    h1 = wk.tile([128, D], F32, tag="h1")
                    nc.vector.tensor_add(h1[:], parts[0][:, 0, :],
                                         parts[0][:, 1, :])
                    h2 = wk.tile([128, D], F32, tag="h2")
                    nc.vector.tensor_add(h2[:], parts[1][:, 0, :],
                                         parts[1][:, 1, :])
                    nc.vector.tensor_add(red[:], h1[:], h2[:])
                elif len(parts) == 1:
                    nc.vector.tensor_add(red[:], parts[0][:, 0, :],
                                         parts[0][:, 1, :])
                else:
                    nc.vector.memset(red[:], 0.0)
                do_tile(t, red)

    nc.compile()
    return nc


def kernel(h_neigh, h_self, src, dst, W_neigh, W_self):
    from concourse.bass_utils import run_bass_kernel_spmd

    in_maps, sched, meta = _prep(h_neigh, h_self, src, dst, W_neigh, W_self)
    key = (str(sched["KAm"]), str(sched["KBm"]))
    if key not in _cache:
        _cache[key] = _build(sched)
    nc = _cache[key]

    import os
    trace = bool(int(os.environ.get("KERNEL_TRACE", "0")))
    res = run_bass_kernel_spmd(nc, in_maps, core_ids=list(range(N_CORES)),
                               trace=trace)
    kernel.last_exec_time_ns = res.exec_time_ns
    kernel.last_result = res

    out = np.zeros((N_NODES, D), dtype=np.float32)
    for c in range(N_CORES):
        nodes = meta["core_nodes"][c]
        dev = res.results[c]["out"]                   # [NPC, 64]
        valid = nodes < N_NODES
        out[nodes[valid]] = dev[valid]
    return out


def last_exec_time_ns():
    return getattr(kernel, "last_exec_time_ns", None)


kernel.last_result = None


# revision 15
# speedup vs baseline: 1.4706x; 1.4706x over previous
"""GNN message-passing (mean aggregation + dual linear + relu + L2 norm)
on 8 Trainium2 NeuronCores.

Strategy (dst-sharded, single-pass, pair-packed gather, 2 overlapping
int16 windows):
  - Nodes are globally sorted by in-degree and dealt round-robin to the 8
    cores (SPMD: identical compiled schedule per core).
  - h_neigh is presented to the device as fp16 PAIR rows: row i =
    [h[2i], h[2i+1]] -> [50000, 128] fp16.  50000 rows exceed the int16
    gather-index range, so two OVERLAPPING row windows are used:
    A = rows [0, 32767), B = rows [17233, 50000).  Pairs in the overlap
    [17233, 32767) are routed per node to balance its A/B slot counts, so
    one shared node order (sorted by the (a, b) slot-count tuple) keeps
    per-tile padding low (~6%) for BOTH windows -> no combine stage, no
    HBM partial bounce.
  - Per 128-node tile: one gather per window fetches [128, K, 128] fp16
    (256-byte descriptors, one per edge-pair-slot).  A per-half fp16 count
    mask zeroes the unwanted half of each pair (and padding); an in-place
    contiguous binary-tree of DVE adds produces the neighbor sums (fp32).
  - Epilogue per tile: scale by 1/deg, PE transpose, dual matmul with
    W_neigh/W_self (PSUM-accumulated), relu, transpose back, fused
    square+row-sum, sqrt, reciprocal, scale, DMA out.
  - CPU does only integer index prep (sorting, routing, dedup) plus input
    layout (fp16 cast/reshape of h_neigh, transpose of h_self, 1/deg) and
    the final row unpermute of the outputs.
"""
import numpy as np
from contextlib import ExitStack

N_NODES = 100000
N_EDGES = 1600000
D = 64
N_CORES = 8
NPC = 12544                 # nodes per core (98 tiles of 128)
NT = NPC // 128             # 98 output tiles per core
PAIRS = 50000               # pair rows in the fp16 table
W_LO = 17233                # window B start (A = [0, 32767), B = [W_LO, 50000))
W_HI = 32767
CALL_COLS = 56              # target gather-call width per window

_cache = {}


def _prep(h_neigh, h_self, src, dst, W_neigh, W_self):
    """CPU-side integer/index preprocessing. Returns (in_maps, sched, meta)."""
    src = np.asarray(src, dtype=np.int64)
    dst = np.asarray(dst, dtype=np.int64)
    h_neigh = np.asarray(h_neigh, dtype=np.float32)
    h_self = np.asarray(h_self, dtype=np.float32)

    deg = np.bincount(dst, minlength=N_NODES)                    # [N]
    order = np.argsort(-deg, kind="stable")                      # degree sort
    n_ext = NPC * N_CORES                                        # 100352
    order_ext = np.concatenate(
        [order, np.full(n_ext - N_NODES, N_NODES, dtype=np.int64)])
    deg_ext = np.concatenate([deg, [0]])
    inv_deg = 1.0 / np.maximum(deg_ext, 1).astype(np.float32)

    rank_of_node = np.empty(N_NODES + 1, dtype=np.int64)
    rank_of_node[order_ext[:n_ext]] = np.arange(n_ext)

    e_rank = rank_of_node[dst]
    e_core = e_rank % N_CORES
    e_pos = e_rank // N_CORES                  # 0..NPC-1
    e_pair = src >> 1
    e_par = src & 1

    tbl16 = np.ascontiguousarray(
        h_neigh.astype(np.float16).reshape(PAIRS, 2 * D))

    # per core: route each edge to window A or B (balance per node using
    # the flexible overlap), dedup (pos, window, relpair) into slots
    per_core = []
    KA = np.zeros((N_CORES, NT), dtype=np.int64)
    KB = np.zeros((N_CORES, NT), dtype=np.int64)
    vorders = []
    for c in range(N_CORES):
        m = e_core == c
        pos = e_pos[m]
        pair = e_pair[m]
        par = e_par[m]
        fa = np.bincount(pos[pair < W_LO], minlength=NPC)
        fl = np.bincount(pos[(pair >= W_LO) & (pair < W_HI)], minlength=NPC)
        d = np.bincount(pos, minlength=NPC)
        na = np.clip((d + 1) // 2, fa, fa + fl)     # A-routed edge count
        # route: forced-A -> A; forced-B -> B; flexible: first (na-fa)
        # per node (in sorted order) -> A
        so = np.lexsort((pair, pos))
        pos_s = pos[so]; pair_s = pair[so]; par_s = par[so]
        flex = (pair_s >= W_LO) & (pair_s < W_HI)
        # rank of each flexible edge within its node's flexible list
        idxf = np.flatnonzero(flex)
        posf = pos_s[idxf]
        first = np.searchsorted(posf, np.arange(NPC), side="left")
        frank = np.arange(idxf.size) - first[posf]
        toA = np.zeros(pos_s.size, dtype=bool)
        toA[pair_s < W_LO] = True
        toA[idxf[frank < (na - fa)[posf]]] = True
        # dedup (pos, window, relpair)
        win = (~toA).astype(np.int64)              # 0 = A, 1 = B
        relp = pair_s - win * W_LO
        key = (pos_s * 2 + win) * PAIRS + relp
        uk, inv = np.unique(key, return_inverse=True)
        cnt = np.zeros((uk.size, 2), dtype=np.int64)
        np.add.at(cnt, (inv, par_s), 1)
        upos = uk // (2 * PAIRS)
        uwin = (uk // PAIRS) % 2
        urel = uk % PAIRS
        a_slots = np.bincount(upos[uwin == 0], minlength=NPC)
        b_slots = np.bincount(upos[uwin == 1], minlength=NPC)
        vo = np.lexsort((-a_slots, -b_slots))
        vorders.append(vo)
        vrank = np.empty(NPC, dtype=np.int64)
        vrank[vo] = np.arange(NPC)
        KA[c] = a_slots[vo].reshape(NT, 128).max(axis=1)
        KB[c] = b_slots[vo].reshape(NT, 128).max(axis=1)
        per_core.append(dict(upos=upos, uwin=uwin, urel=urel, cnt=cnt,
                             vrank=vrank))

    KAm = KA.max(axis=0)
    KBm = KB.max(axis=0)
    SKA = int(KAm.sum())
    SKB = int(KBm.sum())
    colA = np.zeros(NT + 1, dtype=np.int64); colA[1:] = np.cumsum(KAm)
    colB = np.zeros(NT + 1, dtype=np.int64); colB[1:] = np.cumsum(KBm)

    # call grouping: consecutive tiles while BOTH windows fit the cap;
    # taper the final tiles so the tail drain + compute chain is short
    calls = []                      # (t0, t1, colA0, nA, colB0, nB)
    t = 0
    while t < NT:
        cap = CALL_COLS if t < NT - 8 else 28
        t1 = t
        ca = cb = 0
        while t1 < NT and (ca == 0 or
                           (ca + KAm[t1] <= cap and
                            cb + KBm[t1] <= cap)):
            ca += KAm[t1]; cb += KBm[t1]
            t1 += 1
        calls.append((t, t1, int(colA[t]), int(ca), int(colB[t]), int(cb)))
        t = t1
    sched = dict(KAm=KAm.tolist(), KBm=KBm.tolist(), SKA=SKA, SKB=SKB,
                 calls=calls)

    Wn_T = np.ascontiguousarray(W_neigh.astype(np.float32).T)
    Ws_T = np.ascontiguousarray(W_self.astype(np.float32).T)
    wT = np.concatenate([Wn_T, Ws_T], axis=1)                   # [64, 128]
    h_self_ext = np.vstack([h_self, np.zeros((1, D), np.float32)])

    def wrap(M):
        flat = M.T.reshape(-1)
        w = flat.reshape(-1, 16).T
        return np.tile(w, (8, 1)).copy()

    in_maps = []
    core_nodes = []
    for c in range(N_CORES):
        pc = per_core[c]
        upos, uwin, urel, cnt, vrank = (pc["upos"], pc["uwin"], pc["urel"],
                                        pc["cnt"], pc["vrank"])
        vp = vrank[upos]
        part = vp % 128
        tile = vp // 128
        # slot index within (node, window): uk sorted by (pos, win, relp)
        # -> recompute per window
        idxA = np.zeros((128, SKA), dtype=np.int16)
        mskA = np.zeros((128, 2 * SKA), dtype=np.float16)
        idxB = np.zeros((128, SKB), dtype=np.int16)
        mskB = np.zeros((128, 2 * SKB), dtype=np.float16)
        for w, (idxW, mskW, colW) in enumerate(
                ((idxA, mskA, colA), (idxB, mskB, colB))):
            sel = uwin == w
            vpw = vp[sel]; partw = part[sel]; tilew = tile[sel]
            relw = urel[sel]; cntw = cnt[sel]
            # slot rank within node for this window (uk order is sorted by
            # key = (pos*2+win)*PAIRS+relp, so within (pos, win) ascending)
            so = np.lexsort((relw, vpw))
            vpw = vpw[so]; partw = partw[so]; tilew = tilew[so]
            relw = relw[so]; cntw = cntw[so]
            first = np.searchsorted(vpw, np.arange(NPC), side="left")
            srank = np.arange(vpw.size) - first[vpw]
            col = colW[tilew] + srank
            idxW[partw, col] = relw.astype(np.int16)
            mskW[partw, 2 * col] = cntw[:, 0].astype(np.float16)
            mskW[partw, 2 * col + 1] = cntw[:, 1].astype(np.float16)

        nodes = order_ext[c::N_CORES][vorders[c]]
        core_nodes.append(nodes)
        hsT = np.ascontiguousarray(h_self_ext[nodes].T)          # [64, NPC]
        ivd = inv_deg[nodes].reshape(NT, 128).T.copy()           # [128, NT]

        in_maps.append(dict(tbl=tbl16, idxA=wrap(idxA), mskA=mskA,
                            idxB=wrap(idxB), mskB=mskB, hsT=hsT,
                            ivd=ivd, wT=wT))

    meta = dict(core_nodes=core_nodes)
    return in_maps, sched, meta


def _patch_queue_aware_sems():
    """Make Tile's DMASW sem-lane assignment follow each SWDGE instruction's
    queue_num (ucode requires a sem to be updated from a single queue)."""
    from concourse import tile_sem_assignment as tsa
    from concourse import mybir
    if getattr(tsa.TileClockTick, "_qaware_patched", False):
        return
    orig = tsa.TileClockTick._assign_tick

    def _assign_tick_qaware(self, inst):
        qn = getattr(inst, "queue_num", None)
        if qn is not None and getattr(inst, "engine", None) == mybir.EngineType.Pool:
            self.next_sw_dma_idx = int(qn) % self.swdge_sem_count
        return orig(self, inst)

    tsa.TileClockTick._assign_tick = _assign_tick_qaware
    tsa.TileClockTick._qaware_patched = True


def _build(sched):
    import concourse.bacc as bacc
    import concourse.tile as tile
    from concourse import mybir
    from concourse.masks import make_identity

    _patch_queue_aware_sems()

    F32 = mybir.dt.float32
    F16 = mybir.dt.float16
    I16 = mybir.dt.int16
    AF = mybir.ActivationFunctionType
    KAm = np.array(sched["KAm"])
    KBm = np.array(sched["KBm"])
    SKA = int(sched["SKA"]); SKB = int(sched["SKB"])
    calls = sched["calls"]
    colA = np.zeros(NT + 1, dtype=np.int64); colA[1:] = np.cumsum(KAm)
    colB = np.zeros(NT + 1, dtype=np.int64); colB[1:] = np.cumsum(KBm)

    nc = bacc.Bacc("TRN2", target_bir_lowering=False, num_swdge_queues=4,
                   dynamic_dma_scratch_size=32768)
    tbl = nc.declare_dram_parameter("tbl", [PAIRS, 2 * D], F16, isOutput=False)
    idxA = nc.declare_dram_parameter("idxA", [128, SKA * 8], I16, isOutput=False)
    mskA = nc.declare_dram_parameter("mskA", [128, 2 * SKA], F16, isOutput=False)
    idxB = nc.declare_dram_parameter("idxB", [128, SKB * 8], I16, isOutput=False)
    mskB = nc.declare_dram_parameter("mskB", [128, 2 * SKB], F16, isOutput=False)
    hsT = nc.declare_dram_parameter("hsT", [D, NPC], F32, isOutput=False)
    ivd = nc.declare_dram_parameter("ivd", [128, NT], F32, isOutput=False)
    wT = nc.declare_dram_parameter("wT", [D, 2 * D], F32, isOutput=False)
    out = nc.declare_dram_parameter("out", [NPC, D], F32, isOutput=True)

    with tile.TileContext(nc) as tc, ExitStack() as ctx:
        singles = ctx.enter_context(tc.tile_pool(name="singles", bufs=1))
        gpA = ctx.enter_context(tc.tile_pool(name="gathA", bufs=3))
        gpB = ctx.enter_context(tc.tile_pool(name="gathB", bufs=3))
        wk = ctx.enter_context(tc.tile_pool(name="work", bufs=3))
        ps = ctx.enter_context(tc.tile_pool(name="psum", bufs=2, space="PSUM"))

        # split point after call 1: small head loads let gather 0 start
        # immediately; the big remainder loads overlap calls 0-1.  All input
        # loads are issued UPFRONT (before any output DMA is queued) to
        # avoid HWDGE head-of-line blocking.
        n_head = min(2, len(calls))
        hA = calls[n_head - 1][2] + calls[n_head - 1][3] if n_head else 0
        hB = calls[n_head - 1][4] + calls[n_head - 1][5] if n_head else 0

        idxA_h = singles.tile([128, max(hA, 1) * 8], I16)
        nc.sync.dma_start(out=idxA_h[:], in_=idxA[:, 0:max(hA, 1) * 8])
        idxB_h = singles.tile([128, max(hB, 1) * 8], I16)
        nc.sync.dma_start(out=idxB_h[:], in_=idxB[:, 0:max(hB, 1) * 8])
        mskA_h = singles.tile([128, 2 * max(hA, 1)], F16)
        nc.sync.dma_start(out=mskA_h[:], in_=mskA[:, 0:2 * max(hA, 1)])
        mskB_h = singles.tile([128, 2 * max(hB, 1)], F16)
        nc.sync.dma_start(out=mskB_h[:], in_=mskB[:, 0:2 * max(hB, 1)])
        rA = SKA - hA
        rB = SKB - hB
        idxA_r = singles.tile([128, max(rA, 1) * 8], I16)
        nc.sync.dma_start(out=idxA_r[:], in_=idxA[:, hA * 8:(hA + max(rA, 1)) * 8])
        idxB_r = singles.tile([128, max(rB, 1) * 8], I16)
        nc.sync.dma_start(out=idxB_r[:], in_=idxB[:, hB * 8:(hB + max(rB, 1)) * 8])
        mskA_r = singles.tile([128, 2 * max(rA, 1)], F16)
        nc.sync.dma_start(out=mskA_r[:], in_=mskA[:, 2 * hA:2 * (hA + max(rA, 1))])
        mskB_r = singles.tile([128, 2 * max(rB, 1)], F16)
        nc.sync.dma_start(out=mskB_r[:], in_=mskB[:, 2 * hB:2 * (hB + max(rB, 1))])

        # epilogue-side inputs on the Activation HWDGE queue
        ivd_sb = singles.tile([128, NT], F32)
        nc.scalar.dma_start(out=ivd_sb[:], in_=ivd[:])
        wT_sb = singles.tile([D, 2 * D], F32)
        nc.scalar.dma_start(out=wT_sb[:], in_=wT[:])
        hsT_sb = singles.tile([D, NPC], F32)
        nc.scalar.dma_start(out=hsT_sb[:], in_=hsT[:])
        ident = singles.tile([128, 128], F32)
        make_identity(nc, ident[:])
        eps = singles.tile([128, 1], F32)
        nc.gpsimd.memset(eps[:], 1e-30)

        def hs_slice(t):
            return hsT_sb[:, t * 128:(t + 1) * 128]

        def call_views(k):
            """(idx, msk, col-offset) pairs for call k in each window."""
            (t0, t1, cA0, nA, cB0, nB) = calls[k]
            if k < n_head:
                return ((idxA_h, mskA_h, cA0), (idxB_h, mskB_h, cB0))
            return ((idxA_r, mskA_r, cA0 - hA), (idxB_r, mskB_r, cB0 - hB))

        qn = [0]
        def next_q():
            q = qn[0] % 4
            qn[0] += 1
            return q

        def do_tile(t, red):
            agg = wk.tile([128, D], F32, tag="agg")
            nc.scalar.mul(agg[:], red[:], ivd_sb[:, t:t + 1])
            p_aT = ps.tile([D, 128], F32, tag="aT")
            nc.tensor.transpose(out=p_aT[:], in_=agg[:], identity=ident[:])
            aT = wk.tile([D, 128], F32, tag="aTs")
            nc.vector.tensor_copy(aT[:], p_aT[:])
            p_z = ps.tile([D, 128], F32, tag="z")
            nc.tensor.matmul(out=p_z[:], lhsT=wT_sb[:, 0:D], rhs=aT[:],
                             start=True, stop=False)
            nc.tensor.matmul(out=p_z[:], lhsT=wT_sb[:, D:2 * D],
                             rhs=hs_slice(t),
                             start=False, stop=True)
            zT = wk.tile([D, 128], F32, tag="zT")
            nc.scalar.activation(zT[:], p_z[:], AF.Relu)
            p_zn = ps.tile([128, D], F32, tag="zn")
            nc.tensor.transpose(out=p_zn[:], in_=zT[:],
                                identity=ident[0:D, 0:D])
            sq = wk.tile([128, D], F32, tag="sq")
            s = wk.tile([128, 1], F32, tag="s")
            nc.scalar.activation(sq[:], p_zn[:], AF.Square, accum_out=s[:])
            nrm = wk.tile([128, 1], F32, tag="nrm")
            nc.scalar.activation(nrm[:], s[:], AF.Sqrt, bias=eps[:])
            r = wk.tile([128, 1], F32, tag="r")
            nc.vector.reciprocal(r[:], nrm[:])
            o = wk.tile([128, D], F32, tag="o")
            nc.scalar.mul(o[:], p_zn[:], r[:])
            nc.sync.dma_start(out=out[t * 128:(t + 1) * 128, :], in_=o[:])

        def tree(gt, cols2):
            """In-place contiguous binary-tree sum over [128, cols2, 64]
            view `gt`; returns after leaving the total in cols 0..1."""
            cur = cols2
            while cur > 2:
                h = cur // 2
                odd = cur - 2 * h
                nc.vector.tensor_add(
                    gt[:, odd:odd + h, :],
                    gt[:, odd:odd + h, :],
                    gt[:, odd + h:cur, :])
                cur = odd + h
            return cur       # 2 (cols2 >= 2 always) or 1

        for ci, (t0, t1, cA0, nA, cB0, nB) in enumerate(calls):
            (viA, viB) = call_views(ci)
            gA = gB = None
            if nA > 0:
                ia, ma, oA = viA
                gA = gpA.tile([128, nA, 2 * D], F16, tag="gA")
                nc.gpsimd.dma_gather(
                    out_ap=gA[:], in_ap=tbl[0:W_HI, :],
                    idxs_ap=ia[:, oA * 8:(oA + nA) * 8],
                    num_idxs=nA * 128, num_idxs_reg=nA * 128,
                    elem_size=2 * D, single_packet=False,
                    queue_num=next_q())
                gvA = gA[:].rearrange("p c (s d) -> p (c s) d", s=2, d=D)
                mvA = ma[:, 2 * oA:2 * (oA + nA)]
                nc.vector.tensor_mul(
                    gvA, gvA, mvA[:, :, None].broadcast_to((128, 2 * nA, D)))
            if nB > 0:
                ib, mb, oB = viB
                gB = gpB.tile([128, nB, 2 * D], F16, tag="gB")
                nc.gpsimd.dma_gather(
                    out_ap=gB[:], in_ap=tbl[W_LO:PAIRS, :],
                    idxs_ap=ib[:, oB * 8:(oB + nB) * 8],
                    num_idxs=nB * 128, num_idxs_reg=nB * 128,
                    elem_size=2 * D, single_packet=False,
                    queue_num=next_q())
                gvB = gB[:].rearrange("p c (s d) -> p (c s) d", s=2, d=D)
                mvB = mb[:, 2 * oB:2 * (oB + nB)]
                nc.vector.tensor_mul(
                    gvB, gvB, mvB[:, :, None].broadcast_to((128, 2 * nB, D)))
            for t in range(t0, t1):
                ka = int(KAm[t]); kb = int(KBm[t])
                red = wk.tile([128, D], F32, tag="red")
                parts = []
                if ka > 0:
                    la = int(colA[t]) - cA0
                    ga_t = gA[:, la:la + ka, :].rearrange(
                        "p c (s d) -> p (c s) d", s=2, d=D)
                    tree(ga_t, 2 * ka)
                    parts.append(ga_t)
                if kb > 0:
                    lb = int(colB[t]) - cB0
                    gb_t = gB[:, lb:lb + kb, :].rearrange(
                        "p c (s d) -> p (c s) d", s=2, d=D)
                    tree(gb_t, 2 * kb)
                    parts.append(gb_t)
                if len(parts) == 2:
                    h1 = wk.tile([128, D], F32, tag="h1")
                    nc.vector.tensor_add(h1[:], parts[0][:, 0, :],
                                         parts[0][:, 1, :])
                    h2 = wk.tile([128, D], F32, tag="h2")
                    nc.vector.tensor_add(h2[:], parts[1][:, 0, :],
                                         parts[1][:, 1, :])
                    nc.vector.tensor_add(red[:], h1[:], h2[:])
                elif len(parts) == 1:
                    nc.vector.tensor_add(red[:], parts[0][:, 0, :],
                                         parts[0][:, 1, :])
                else:
                    nc.vector.memset(red[:], 0.0)
                do_tile(t, red)

    nc.compile()
    return nc


def kernel(h_neigh, h_self, src, dst, W_neigh, W_self):
    from concourse.bass_utils import run_bass_kernel_spmd

    in_maps, sched, meta = _prep(h_neigh, h_self, src, dst, W_neigh, W_self)
    key = (str(sched["KAm"]), str(sched["KBm"]))
    if key not in _cache:
        _cache[key] = _build(sched)
    nc = _cache[key]

    import os
    trace = bool(int(os.environ.get("KERNEL_TRACE", "0")))
    res = run_bass_kernel_spmd(nc, in_maps, core_ids=list(range(N_CORES)),
                               trace=trace)
    kernel.last_exec_time_ns = res.exec_time_ns
    kernel.last_result = res

    out = np.zeros((N_NODES, D), dtype=np.float32)
    for c in range(N_CORES):
        nodes = meta["core_nodes"][c]
        dev = res.results[c]["out"]                   # [NPC, 64]
        valid = nodes < N_NODES
        out[nodes[valid]] = dev[valid]
    return out


def last_exec_time_ns():
    return getattr(kernel, "last_exec_time_ns", None)


kernel.last_result = None
